# revision 15
# baseline (speedup 1.0000x reference)
"""Multi-head causal self-attention (no RoPE) on 8 Trainium2 NeuronCores.

Problem: x[4,2048,1024], 16 heads x 64 dim, causal softmax, fp32 in/out.

Sharding: DP over batch (4) x TP over head-groups (2 x 8 heads) = 8 cores,
no cross-core collectives. Host sums the two TP partials per batch.

Per-core design (v2 — bf16 + software-pipelined unified schedule):
  - All matmul operands bf16 (PSUM accumulates f32). bf16 runs the PE at
    1 cycle/row at ANY width (f32r needs >=256), halves DMA bytes, and
    speeds 16-bit DVE ops. rel err ~1e-3 << 2e-2 tolerance.
  - Transposed flash attention: scoresT [k,q] blocks so PV consumes probsT
    directly. Softmax without max-subtraction; denominators via a ones
    column appended to V (PV outputs 65 rows). Score pairs run
    concurrently in the PE via row groups (base partitions 0/64).
  - The exp stream on the scalar engine (~150us total) is the serial
    backbone: all 160 (q-tile, head-pair, k-tile) iterations form one
    pipeline, with the PV pair trailing the score pair by one k-tile so
    the PE never waits on exp. QKV projections for slice i+1 and output
    projections WO(i) are emitted as "filler" matmul chains between
    attention iterations, paced by a deficit counter with per-chain
    deadlines, keeping the tensor engine dense (HAM clock stays 2.4 GHz)
    while exp runs back-to-back.
  - Causal mask: gpsimd affine_select on only the 128-wide diagonal
    subblock; score/exp/PV widths are exact (no flooring needed in bf16).
  - Output in bf16, staged per q-tile and shipped as batched DMAs; host
    upcasts, sums TP partials, transposes.

Self-contained: hardcodes all shapes; builds + compiles the Bass program
once per process and reuses it.
"""
import numpy as np
import ml_dtypes

import concourse.bass as bass  # noqa: F401  (engine namespaces live on nc)
import concourse.mybir as mybir
from concourse import bacc
from concourse.tile import TileContext
from concourse import bass_utils

F32 = mybir.dt.float32
BF16 = mybir.dt.bfloat16
EXP = mybir.ActivationFunctionType.Exp

B, S, D = 4, 2048, 1024
H, HD = 16, 64
TP = 2                  # head-group (tensor parallel) factor
HLOC = H // TP          # 8 heads per core
DLOC = HLOC * HD        # 512 attn dims per core
P = 128                 # partition tile
NQ = 512                # q-tile width (seq)
NQT = S // NQ           # 4 q-tiles
KD = D // P             # 8 contraction tiles over d_model
MD = DLOC // P          # 4 head-pairs (dloc m-tiles)
VW = HLOC * (HD + 1)    # 520: v row width, ones column per head

# global attention-iteration index at which each phase starts
PHASE_START = [0, 16, 48, 96, 160]

_NC = None


def _exp_ns(nw):
    return (2 * nw + 352) / 1.2


def _iter_pe_ns(nw):
    # concurrent score pair (~nw/2.4) + two serial PV matmuls + dispatch
    return 3 * nw / 2.4 + 150.0


def _build():
    nc = bacc.Bacc("TRN2", target_bir_lowering=False, debug=False)
    xT = nc.dram_tensor("xT", [D, S], BF16, kind="ExternalInput").ap()
    wqT = nc.dram_tensor("wqT", [D, DLOC], BF16, kind="ExternalInput").ap()
    wkT = nc.dram_tensor("wkT", [D, DLOC], BF16, kind="ExternalInput").ap()
    wvT = nc.dram_tensor("wvT", [D, DLOC], BF16, kind="ExternalInput").ap()
    woT = nc.dram_tensor("woT", [DLOC, D], BF16, kind="ExternalInput").ap()
    outT = nc.dram_tensor("outT", [D, S], BF16, kind="ExternalOutput").ap()

    with TileContext(nc) as tc:
        with tc.tile_pool(name="wpool", bufs=1) as wpool, \
             tc.tile_pool(name="xpool", bufs=1) as xpool, \
             tc.tile_pool(name="kvpool", bufs=1) as kvpool, \
             tc.tile_pool(name="qpool", bufs=1) as qpool, \
             tc.tile_pool(name="ppool", bufs=4) as ppool, \
             tc.tile_pool(name="apool", bufs=1) as apool, \
             tc.tile_pool(name="spool", bufs=1) as spool, \
             tc.tile_pool(name="psum", bufs=2, space="PSUM") as psum:

            # warm the ACT exp table while weight DMAs run
            warm_in = spool.tile([P, 8], F32, name="warm_in", tag="warm_in")
            nc.vector.memset(warm_in, 0.0)
            warm = spool.tile([P, 8], F32, name="warm", tag="warm")
            nc.scalar.activation(warm, warm_in, EXP)

            # ---------------- weight / x preloads ----------------
            # wq first (gates the very first matmul chain), in two halves
            wq_sb = [None] * KD
            for h in range(2):
                t = wpool.tile([P, 4 * DLOC], BF16, name=f"wqb{h}")
                tv_ = t.rearrange("p (k e) -> p k e", k=4)
                nc.scalar.dma_start(
                    tv_,
                    wqT.rearrange("(k p) e -> p k e", p=P)[:, 4 * h:4 * h + 4, :])
                for k in range(4):
                    wq_sb[4 * h + k] = tv_[:, k, :]
            tk = wpool.tile([P, KD * DLOC], BF16, name="wkb")
            tkv = tk.rearrange("p (k e) -> p k e", k=KD)
            nc.gpsimd.dma_start(tkv, wkT.rearrange("(k p) e -> p k e", p=P))
            wk_sb = [tkv[:, k, :] for k in range(KD)]
            tv = wpool.tile([P, KD * DLOC], BF16, name="wvb")
            tvv = tv.rearrange("p (k e) -> p k e", k=KD)
            nc.gpsimd.dma_start(tvv, wvT.rearrange("(k p) e -> p k e", p=P))
            wv_sb = [tvv[:, k, :] for k in range(KD)]
            to = wpool.tile([P, MD * D], BF16, name="wob")
            tov = to.rearrange("p (d e) -> p d e", d=MD)
            nc.scalar.dma_start(tov, woT.rearrange("(d p) e -> p d e", p=P))
            wo_sb = [tov[:, d, :] for d in range(MD)]

            x_view = {}

            def load_x(i):
                if i == 0:
                    xts = []
                    for k in range(KD):
                        t = xpool.tile([P, NQ], BF16, name=f"xa{k}",
                                       tag=f"xa{k}")
                        nc.sync.dma_start(t, xT[k * P:(k + 1) * P, 0:NQ])
                        xts.append(t)
                    x_view[0] = xts
                    return
                t = xpool.tile([P, KD * NQ], BF16, name=f"xb{i}", tag="xb",
                               bufs=2)
                tv_ = t.rearrange("p (k s) -> p k s", k=KD)
                nc.sync.dma_start(
                    tv_, xT[:, i * NQ:(i + 1) * NQ]
                    .rearrange("(k p) s -> p k s", p=P))
                x_view[i] = [tv_[:, k, :] for k in range(KD)]

            load_x(0)

            k_sb = {}        # (hp, slice) -> kT tile [128 pair-dims, 512]
            v_sb = {}        # seq tile -> v tile [128 seq, 520]
            q_tiles = {}     # slice -> list of 4 q tiles
            attn_tiles = {}  # (i, hp) -> attn tile [128, 512] bf16
            so_tiles = {}    # i -> output staging tile

            # ---------------- chain builders (PE filler units) ----------
            def q_chain(i, hp):
                def emit():
                    with nc.named_scope(f"q{i}_{hp}"):
                        ps = psum.tile([P, NQ], F32, name=f"psq{i}{hp}",
                                       tag="big")
                        xts = x_view[i]
                        for k in range(KD):
                            nc.tensor.matmul(
                                ps, wq_sb[k][:, hp * P:(hp + 1) * P], xts[k],
                                start=(k == 0), stop=(k == KD - 1))
                        qt = qpool.tile([P, NQ], BF16, name=f"q{hp}",
                                        tag=f"q{hp}", bufs=2)
                        nc.vector.tensor_copy(qt, ps)
                        q_tiles.setdefault(i, [None] * MD)[hp] = qt
                return emit

            def k_chain(i, hp):
                def emit():
                    with nc.named_scope(f"k{i}_{hp}"):
                        ps = psum.tile([P, NQ], F32, name=f"psk{i}{hp}",
                                       tag="big")
                        xts = x_view[i]
                        for k in range(KD):
                            nc.tensor.matmul(
                                ps, wk_sb[k][:, hp * P:(hp + 1) * P], xts[k],
                                start=(k == 0), stop=(k == KD - 1))
                        kt_t = kvpool.tile([P, NQ], BF16, name=f"k{hp}_{i}")
                        nc.vector.tensor_copy(kt_t, ps)
                        k_sb[(hp, i)] = kt_t
                return emit

            def v_chain(i, s_):
                def emit():
                    ti = i * (NQ // P) + s_
                    with nc.named_scope(f"v{ti}"):
                        ps = psum.tile([P, DLOC], F32, name=f"psv{ti}",
                                       tag="big")
                        xts = x_view[i]
                        for k in range(KD):
                            nc.tensor.matmul(
                                ps, xts[k][:, s_ * P:(s_ + 1) * P], wv_sb[k],
                                start=(k == 0), stop=(k == KD - 1))
                        vt = kvpool.tile([P, VW], BF16, name=f"v{ti}")
                        vr = vt.rearrange("p (h c) -> p h c", c=HD + 1)
                        # contiguous memset fills the per-head ones columns;
                        # the strided copy then overwrites the data columns
                        # (strided memset is unreliable on HW)
                        nc.vector.memset(vt, 1.0)
                        nc.vector.tensor_copy(
                            vr[:, :, 0:HD],
                            ps.rearrange("p (h d) -> p h d", d=HD))
                        v_sb[ti] = vt
                return emit

            def store_out(i):
                so = so_tiles[i]
                sov = so.rearrange("p (e q) -> p e q", e=KD)
                dst = outT[:, i * NQ:(i + 1) * NQ].rearrange(
                    "(e p) q -> p e q", p=P)
                nc.sync.dma_start(dst, sov)

            def wo_chain(i, e, tag="big"):
                def emit():
                    with nc.named_scope(f"wo{i}_{e}"):
                        ps = psum.tile([P, NQ], F32, name=f"pso{i}{e}",
                                       tag=tag)
                        for d in range(MD):
                            nc.tensor.matmul(
                                ps, wo_sb[d][:, e * P:(e + 1) * P],
                                attn_tiles[(i, d)], start=(d == 0),
                                stop=(d == MD - 1))
                        so = so_tiles[i]
                        sov = so.rearrange("p (e q) -> p e q", e=KD)[:, e, :]
                        nc.vector.tensor_copy(sov, ps)
                        if e == KD - 1 and i < NQT - 1:
                            store_out(i)
                return emit

            # ---------------- filler scheduler ----------------
            # items: [cost_ns, deadline_iter, min_iter, emit_fn]
            fillers = []
            giter = [0]
            deficit = [0.0]

            QKV_COST = 8 * (NQ / 2.4) + 250.0
            WO_COST = 4 * (NQ / 2.4) + 250.0

            def pump():
                while True:
                    idx = None
                    for j, f in enumerate(fillers):
                        if f[2] > giter[0]:
                            continue  # not eligible yet (e.g. wo chains
                            # waiting out the prior phase's last normalize)
                        if deficit[0] >= f[0] or f[1] <= giter[0] + 2:
                            idx = j
                            break
                    if idx is None:
                        return
                    cost, _, _, fn = fillers.pop(idx)
                    fn()
                    deficit[0] -= cost

            def drain(upto_deadline):
                while fillers and fillers[0][1] <= upto_deadline:
                    cost, _, _, fn = fillers.pop(0)
                    fn()
                    deficit[0] -= cost

            def push_qkv(j):
                ph = PHASE_START[j]
                nkt = 4 * (j + 1)
                for hp in range(MD):
                    fillers.append([QKV_COST, ph, 0, q_chain(j, hp)])
                fillers.append([QKV_COST, ph + 4 * j, 0, k_chain(j, 0)])
                for s_ in range(NQ // P):
                    fillers.append([QKV_COST, ph + 4 * j, 0, v_chain(j, s_)])
                for hp in range(1, MD):
                    fillers.append(
                        [QKV_COST, ph + hp * nkt + 4 * j, 0, k_chain(j, hp)])
                fillers.sort(key=lambda f: f[1])

            def push_wo(i):
                so_tiles[i] = spool.tile([P, KD * NQ], BF16, name=f"so{i}",
                                         tag="so", bufs=2)
                # wo(i) reads attn(i, hp3), whose normalize completes a few
                # iterations into the NEXT phase — emitting earlier would
                # head-of-line-block the PE FIFO on that dependency
                nb = PHASE_START[i + 1] + 6
                for e in range(KD):
                    fillers.append(
                        [WO_COST, PHASE_START[NQT], nb, wo_chain(i, e)])
                fillers.sort(key=lambda f: f[1])

            # ---------------- qkv slice 0 (dense head) ----------------
            with nc.named_scope("qkv0"):
                q_chain(0, 0)()
                k_chain(0, 0)()
                for s_ in range(NQ // P):
                    v_chain(0, s_)()
                for hp in range(1, MD):
                    q_chain(0, hp)()
                    k_chain(0, hp)()
            load_x(1)

            # ---------------- unified attention pipeline ----------------
            for i in range(NQT):
                nkt = 4 * (i + 1)
                if i + 1 < NQT:
                    push_qkv(i + 1)
                if i + 2 < NQT:
                    load_x(i + 2)
                if i >= 1:
                    push_wo(i - 1)

                with nc.named_scope(f"attn{i}"):
                    for hp in range(MD):
                        q_cur = q_tiles[i][hp]
                        pvA = psum.tile([HD + 1, NQ], F32,
                                        name=f"pvA{i}_{hp}", tag="pv")
                        pvB = psum.tile([HD + 1, NQ], F32,
                                        name=f"pvB{i}_{hp}", tag="pv")
                        pending_pv = None  # (kt, c0, pp)

                        def flush_pv(hp=hp, pvA=pvA, pvB=pvB, nkt=nkt):
                            nonlocal pending_pv
                            if pending_pv is None:
                                return
                            kt, c0, pp = pending_pv
                            vt = v_sb[kt]
                            hA, hB = 2 * hp, 2 * hp + 1
                            nc.tensor.matmul(
                                pvA[:, c0:NQ],
                                vt[:, hA * (HD + 1):(hA + 1) * (HD + 1)],
                                pp[:, c0:NQ],
                                start=(kt == 0), stop=(kt == nkt - 1))
                            nc.tensor.matmul(
                                pvB[:, c0:NQ],
                                vt[:, hB * (HD + 1):(hB + 1) * (HD + 1)],
                                pp[:, NQ + c0:2 * NQ],
                                start=(kt == 0), stop=(kt == nkt - 1))
                            pending_pv = None

                        for kt in range(nkt):
                            st, col = divmod(kt, 4)
                            ksl = k_sb[(hp, st)]
                            r = kt - 4 * i
                            c0 = 0 if r < 0 else r * P
                            nw = NQ - c0
                            sc = psum.tile([P, 2 * NQ], F32,
                                           name=f"sc{i}{hp}{kt}", tag="sc")
                            nc.tensor.matmul(
                                sc[:, c0:NQ],
                                ksl[0:HD, col * P:(col + 1) * P],
                                q_cur[0:HD, c0:NQ],
                                start=True, stop=True)
                            nc.tensor.matmul(
                                sc[:, NQ + c0:2 * NQ],
                                ksl[HD:P, col * P:(col + 1) * P],
                                q_cur[HD:P, c0:NQ],
                                start=True, stop=True)
                            pp = ppool.tile([P, 2 * NQ], BF16, name="pp",
                                            tag="pp")
                            scv = sc.rearrange("p (h q) -> p h q", q=NQ)
                            ppv = pp.rearrange("p (h q) -> p h q", q=NQ)
                            nc.scalar.activation(ppv[:, :, c0:NQ],
                                                 scv[:, :, c0:NQ], EXP)
                            if r >= 0:
                                # causal mask, 128-wide diagonal block only
                                nc.gpsimd.affine_select(
                                    out=ppv[:, :, c0:c0 + P],
                                    in_=ppv[:, :, c0:c0 + P],
                                    compare_op=mybir.AluOpType.is_ge,
                                    fill=0.0, base=0,
                                    pattern=[[0, 2], [1, P]],
                                    channel_multiplier=-1)
                            giter[0] += 1
                            deficit[0] += _exp_ns(nw) - _iter_pe_ns(nw)
                            pump()
                            flush_pv()
                            pending_pv = (kt, c0, pp)
                            pump()
                        flush_pv()

                        # ---- normalize: attn[d,q] = pv[d,q] / pv[64,q] ----
                        with nc.named_scope(f"norm{i}_{hp}"):
                            pvsA = spool.tile([HD, NQ], F32, name="pvsA",
                                              tag="pvsA", bufs=2)
                            pvsB = spool.tile([HD, NQ], F32, name="pvsB",
                                              tag="pvsB", bufs=2)
                            dnA = spool.tile([1, NQ], F32, name="dnA",
                                             tag="dnA", bufs=2)
                            dnB = spool.tile([1, NQ], F32, name="dnB",
                                             tag="dnB", bufs=2)
                            # last hp: copies on ACT (idle at the phase
                            # boundary) so they don't queue behind the DVE
                            # FIFO and stall filler-chain PSUM releases
                            ce = nc.scalar.copy if hp == MD - 1 \
                                else nc.vector.tensor_copy
                            ce(pvsA, pvA[0:HD, :])
                            ce(dnA, pvA[HD:HD + 1, :])
                            ce(pvsB, pvB[0:HD, :])
                            ce(dnB, pvB[HD:HD + 1, :])
                            rcA = spool.tile([1, NQ], F32, name="rcA",
                                             tag="rcA", bufs=2)
                            rcB = spool.tile([1, NQ], F32, name="rcB",
                                             tag="rcB", bufs=2)
                            nc.vector.reciprocal_approx_fast(rcA, dnA)
                            nc.vector.reciprocal_approx_fast(rcB, dnB)
                            # separate base-0 bc tiles: partition_broadcast
                            # to a non-zero out base partition misbehaves
                            bcA = spool.tile([HD, NQ], F32, name="bcA",
                                             tag="bcA", bufs=2)
                            bcB = spool.tile([HD, NQ], F32, name="bcB",
                                             tag="bcB", bufs=2)
                            nc.gpsimd.partition_broadcast(bcA, rcA)
                            nc.gpsimd.partition_broadcast(bcB, rcB)
                            at = apool.tile([P, NQ], BF16, name=f"attn{hp}",
                                            tag=f"attn{hp}", bufs=2)
                            nc.vector.tensor_mul(at[0:HD, :], pvsA, bcA)
                            nc.vector.tensor_mul(at[HD:P, :], pvsB, bcB)
                            attn_tiles[(i, hp)] = at

                # force chains attn(i+1) depends on; leftovers stay queued
                drain(PHASE_START[i + 1])
                # replenish filler budget: the boundary bubble (last hp's
                # normalize serializes ~5us) wants immediate PE work
                deficit[0] = max(deficit[0], 2.2 * QKV_COST)

            # ---------------- final output projection + store ----------
            with nc.named_scope("wo3"):
                so_tiles[NQT - 1] = spool.tile([P, KD * NQ], BF16, name="so3",
                                               tag="so", bufs=2)
                drain(PHASE_START[NQT] + 1)
                for e in range(KD):
                    wo_chain(NQT - 1, e, tag=("sc" if e % 2 else "big"))()
                    if e == 3:
                        so = so_tiles[NQT - 1]
                        nc.sync.dma_start(
                            outT[:, (NQT - 1) * NQ:].rearrange(
                                "(e p) q -> p e q", p=P)[:, 0:4, :],
                            so.rearrange("p (e q) -> p e q", e=KD)[:, 0:4, :])
                so = so_tiles[NQT - 1]
                nc.sync.dma_start(
                    outT[:, (NQT - 1) * NQ:].rearrange(
                        "(e p) q -> p e q", p=P)[:, 4:8, :],
                    so.rearrange("p (e q) -> p e q", e=KD)[:, 4:8, :])
    nc.compile()
    return nc


def _get_nc():
    global _NC
    if _NC is None:
        _NC = _build()
    return _NC


def make_in_maps(x, w_q, w_k, w_v, w_o):
    bf = ml_dtypes.bfloat16
    x = np.asarray(x, np.float32)
    w_q = np.asarray(w_q, np.float32) * (1.0 / np.sqrt(HD))
    w_k = np.asarray(w_k, np.float32)
    w_v = np.asarray(w_v, np.float32)
    w_o = np.asarray(w_o, np.float32)
    in_maps = []
    for c in range(B * TP):
        b, g = divmod(c, TP)
        hsl = slice(g * DLOC, (g + 1) * DLOC)
        in_maps.append({
            "xT": np.ascontiguousarray(x[b].T).astype(bf),
            "wqT": np.ascontiguousarray(w_q[hsl].T).astype(bf),
            "wkT": np.ascontiguousarray(w_k[hsl].T).astype(bf),
            "wvT": np.ascontiguousarray(w_v[hsl].T).astype(bf),
            "woT": np.ascontiguousarray(w_o[:, hsl].T).astype(bf),
        })
    return in_maps


def gather_out(results):
    out = np.empty((B, S, D), np.float32)
    for b in range(B):
        acc = (results[TP * b]["outT"].astype(np.float32)
               + results[TP * b + 1]["outT"].astype(np.float32))
        out[b] = acc.T
    return out


def kernel(x, w_q, w_k, w_v, w_o):
    nc = _get_nc()
    in_maps = make_in_maps(x, w_q, w_k, w_v, w_o)
    res = bass_utils.run_bass_kernel_spmd(nc, in_maps,
                                          core_ids=list(range(B * TP)))
    return gather_out(res.results)
